# revision 16
# baseline (speedup 1.0000x reference)
"""Trainium2 Bass kernel for nn_DualInt8LinearConv.

Reference computation (N=8192, Cin=4096, Cout=4096):
    x2d      = x.reshape(N, Cin)
    amax     = max(|x2d|, axis=1)                      # [N]
    scale_x  = softplus(amax) / 32767                  # [N]
    xq       = round(x2d / scale_x)                    # [N, Cin]  (never clipped:
                                                       #  softplus(a) > a => |x|/scale < 32767)
    y        = (xq@w0.T * s0 + xq@w1.T * s1) * scale_x + bias

Algebraic collapse: with Wd = s0[:,None]*w0 + s1[:,None]*w1,
    y = scale_x * (xq @ Wd.T) + bias
      = (x + scale_x*eps) @ Wd.T + bias          (eps = rounding, |eps| <= 0.5)
      ~ x @ Wd.T + bias
The xq rounding term contributes ~4e-5 absolute (vs |y|max ~7.3), i.e. the whole
fake-quant pipeline is a single GEMM to well below the 2e-2 gate. Casting x and
Wd to bf16 gives measured scale-relative error 2.4e-3.

Strategy:
  * Row-shard N across 8 cores (1024 rows each); Wd replicated.
  * Host: Wd in fp32, pre-tiled to [n_og, 128(cin sub), n_ct*128(cout)] bf16;
    x shard pre-transposed to [Cin, n_shard] and cast bf16 (halves DMA).
  * Device: x resident in SBUF [128, n_ct, n_shard] bf16 (64KB/partition).
    Per 128-Cout group og: stream Wd (1MB DMA per og), accumulate
    psum[128, n_shard] over the 32 Cin tiles (2 matmuls of N=512 per tile),
    epilogue = single DVE op (+bias into SBUF f32), DMA out.
  * DMA split (v4): x as 32 per-ct DMAs on gpsimd — the SWDGE queue has no
    framework preamble, so x issues at ~1us and paces og0 ct-by-ct (the
    sync/scalar HWDGE queues run ~6us of semaphore-init first); weights on
    sync (1MB per og, bufs=4); y + bias on scalar (HWDGE ring 2, keeps the
    output drain off the x path). og0/og1 are interleaved at 4-ct
    granularity so the PE has ~27.7us of work available while the x load
    completes; the last og runs as two half-width passes so its
    epilogue+store overlaps its own matmuls.
  * Tensor work: 2048 matmuls @ N=512 bf16 ~= 443us/core; everything else
    overlaps behind it.
"""

import os
import sys

sys.path.insert(0, "/opt/trn_rl_repo")

from contextlib import ExitStack

import numpy as np
import ml_dtypes

import concourse.bass as bass
import concourse.mybir as mybir
from concourse import bacc
import concourse.tile as tile

F32 = mybir.dt.float32
BF16 = mybir.dt.bfloat16

N_FULL, CIN, COUT = 8192, 4096, 4096
NCORES = 8


def build_nc(n_shard=N_FULL // NCORES, cin=CIN, cout=COUT):
    n_ct = cin // 128       # Cin tiles (contraction)
    n_og = cout // 128      # Cout groups
    HB = min(512, n_shard)  # matmul moving-operand width
    NH = n_shard // HB
    XC = min(4, n_ct)       # cts per 1MB x-load chunk
    NXC = n_ct // XC

    nc = bacc.Bacc()
    # x shard in DRAM already in the SBUF-resident layout [128, n_ct, n_shard]:
    # element [p, ct, k] = xT[ct*128 + p, k]; chunk DMAs are then
    # stride-identical on both sides.
    xtb = nc.declare_dram_parameter(
        "xtb", [128, n_ct, n_shard], BF16, isOutput=False
    )
    wh = nc.declare_dram_parameter("wh", [n_og, 128, n_ct * 128], BF16, isOutput=False)
    bp = nc.declare_dram_parameter("bp", [128, n_og], F32, isOutput=False)
    yt = nc.declare_dram_parameter("yt", [cout, n_shard], F32, isOutput=True)

    with tile.TileContext(nc) as tc, ExitStack() as ctx:
        p_xall = ctx.enter_context(tc.tile_pool(name="xall", bufs=1))
        p_w = ctx.enter_context(tc.tile_pool(name="wts", bufs=4))
        p_out = ctx.enter_context(tc.tile_pool(name="outs", bufs=4))
        p_s = ctx.enter_context(tc.tile_pool(name="svec", bufs=1))

        ball = p_s.tile([128, n_og], F32)
        nc.scalar.dma_start(ball, bp[:])

        # HAM warmup: the PE idles ~7-14us anyway while the NEFF preamble
        # runs and the first x/weight bytes land; dummy matmuls there keep
        # the PE_HAM activity window busy so the real stream starts (and
        # stays) at K=8/8 = 2.4GHz instead of paying ~5us of cold-clock.
        wz = p_s.tile([128, 512], BF16)
        nc.vector.memset(wz, 0.0)
        with tc.tile_pool(name="warm", bufs=1, space="PSUM") as p_wps:
            wps = p_wps.tile([128, 512], F32)
            for _ in range(24):
                nc.tensor.matmul(wps, wz[:, 0:128], wz, start=True, stop=True)

        # x resident in SBUF as bf16, contraction dim on partitions;
        # per-ct 256KB DMAs on gpsimd (SWDGE issues immediately, fine-grained
        # pacing for the og0/og1 interleave)
        xall = p_xall.tile([128, n_ct, n_shard], BF16)
        for ct in range(n_ct):
            nc.gpsimd.dma_start(xall[:, ct, :], xtb[:, ct, :])

        # weights: sync HWDGE ring, bufs=4 caps early prefetch at 4MB.
        # og0/og1 load as 256KB quarters so the very first matmul only waits
        # on a small transfer (the first MM gates the whole stream); the
        # steady-state ogs use a single efficient 1MB DMA.
        wt_all = []
        for og in range(n_og):
            wt_ = p_w.tile([128, n_ct * 128], BF16, tag="w")
            if og < 2:
                q = n_ct * 128 // 4
                for j in range(4):
                    nc.sync.dma_start(
                        wt_[:, j * q:(j + 1) * q], wh[og, :, j * q:(j + 1) * q]
                    )
            else:
                nc.sync.dma_start(wt_, wh[og, :, :])
            wt_all.append(wt_)

        def mm_og(p_ps, og, cts, ps=None, nbs=range(NH)):
            if ps is None:
                ps = p_ps.tile([128, n_shard], F32, tag="ps")
            for ct in cts:
                lhs = wt_all[og][:, ct * 128:(ct + 1) * 128]
                first, last = ct == 0, ct == n_ct - 1
                for nb in nbs:
                    ns = slice(nb * HB, (nb + 1) * HB)
                    nc.tensor.matmul(
                        ps[:, ns], lhs, xall[:, ct, ns],
                        start=first, stop=last,
                    )
            return ps

        def epilogue(og, ps, nb=None):
            if nb is None:
                t1 = p_out.tile([128, n_shard], F32, tag="t1")
                nc.vector.tensor_scalar_add(t1, ps, ball[:, og:og + 1])
                nc.scalar.dma_start(yt[og * 128:(og + 1) * 128, :], t1)
            else:
                ns = slice(nb * HB, (nb + 1) * HB)
                t1 = p_out.tile([128, HB], F32, tag="t1h")
                nc.vector.tensor_scalar_add(t1, ps[:, ns], ball[:, og:og + 1])
                nc.scalar.dma_start(yt[og * 128:(og + 1) * 128, ns], t1)

        with tc.tile_pool(name="ps", bufs=4, space="PSUM") as p_ps:
            # first pass: og0+og1 interleaved at x-chunk granularity, so the
            # PE has 2 ogs of work to chew on while x streams in
            ps01 = [None, None]
            for c in range(NXC):
                cts = range(c * XC, (c + 1) * XC)
                for og in (0, 1):
                    ps01[og] = mm_og(p_ps, og, cts, ps01[og])
            for og in (0, 1):
                epilogue(og, ps01[og])

            for og in range(2, n_og - 1):
                ps = mm_og(p_ps, og, range(n_ct))
                epilogue(og, ps)

            # last og: two half-width passes so epilogue+store of the first
            # half overlaps the second half's matmuls (shorter kernel tail)
            og = n_og - 1
            ps = p_ps.tile([128, n_shard], F32, tag="ps")
            for nb in range(NH):
                mm_og(p_ps, og, range(n_ct), ps, nbs=(nb,))
                epilogue(og, ps, nb=nb)

    nc.finalize()
    return nc


def _prep_weights(w, n_og, n_ct):
    # [cout, cin] -> [n_og, 128(p=cin sub), n_ct*128(o)] where
    # out[og, p, ct*128+o] = w[og*128+o, ct*128+p]
    cout, cin = w.shape
    wr = w.reshape(n_og, 128, n_ct, 128)        # [og, o, ct, p]
    wr = wr.transpose(0, 3, 2, 1)               # [og, p, ct, o]
    return np.ascontiguousarray(wr.reshape(n_og, 128, n_ct * 128)).astype(
        ml_dtypes.bfloat16
    )


def _prep_svec(v, n_og):
    # [cout] -> [128, n_og] with v[og*128 + p] at [p, og]
    return np.ascontiguousarray(
        np.asarray(v, np.float32).reshape(n_og, 128).T
    )


def kernel(x, w0, w1, s0, s1, bias):
    from concourse.bass_utils import run_bass_kernel_spmd

    N, Cin = x.shape[0], x.shape[1]
    Cout = w0.shape[0]
    n_shard = N // NCORES
    n_ct = Cin // 128
    n_og = Cout // 128

    x2d = np.asarray(x, dtype=np.float32).reshape(N, Cin)
    Wd = (
        np.asarray(s0, np.float32)[:, None] * np.asarray(w0, np.float32)
        + np.asarray(s1, np.float32)[:, None] * np.asarray(w1, np.float32)
    )

    whp = _prep_weights(Wd, n_og, n_ct)
    bpp = _prep_svec(bias, n_og)

    nc = build_nc(n_shard=n_shard, cin=Cin, cout=Cout)

    in_maps = []
    for k in range(NCORES):
        xtk = x2d[k * n_shard:(k + 1) * n_shard].T  # [Cin, n_shard]
        xtk = xtk.reshape(n_ct, 128, n_shard).transpose(1, 0, 2)
        xtk = np.ascontiguousarray(xtk).astype(ml_dtypes.bfloat16)
        in_maps.append({"xtb": xtk, "wh": whp, "bp": bpp})

    res = run_bass_kernel_spmd(
        nc,
        in_maps,
        core_ids=list(range(NCORES)),
        trace=bool(int(os.environ.get("KERNEL_TRACE", "0"))),
    )

    y = np.empty((N, Cout), dtype=np.float32)
    for k in range(NCORES):
        y[k * n_shard:(k + 1) * n_shard] = res.results[k]["yt"].T
    out = y.reshape(N, Cout, 1, 1)
    kernel.last_results = res
    return out


# revision 20
# speedup vs baseline: 1.0005x; 1.0005x over previous
"""Trainium2 Bass kernel for nn_DualInt8LinearConv.

Reference computation (N=8192, Cin=4096, Cout=4096):
    x2d      = x.reshape(N, Cin)
    amax     = max(|x2d|, axis=1)                      # [N]
    scale_x  = softplus(amax) / 32767                  # [N]
    xq       = round(x2d / scale_x)                    # [N, Cin]  (never clipped:
                                                       #  softplus(a) > a => |x|/scale < 32767)
    y        = (xq@w0.T * s0 + xq@w1.T * s1) * scale_x + bias

Algebraic collapse: with Wd = s0[:,None]*w0 + s1[:,None]*w1,
    y = scale_x * (xq @ Wd.T) + bias
      = (x + scale_x*eps) @ Wd.T + bias          (eps = rounding, |eps| <= 0.5)
      ~ x @ Wd.T + bias
The xq rounding term contributes ~4e-5 absolute (vs |y|max ~7.3), i.e. the whole
fake-quant pipeline is a single GEMM to well below the 2e-2 gate. Casting x and
Wd to bf16 gives measured scale-relative error 2.4e-3.

Strategy:
  * Row-shard N across 8 cores (1024 rows each); Wd replicated.
  * Host: Wd in fp32, pre-tiled to [n_og, 128(cin sub), n_ct*128(cout)] bf16;
    x shard pre-transposed to [Cin, n_shard] and cast bf16 (halves DMA).
  * Device: x resident in SBUF [128, n_ct, n_shard] bf16 (64KB/partition).
    Per 128-Cout group og: stream Wd (1MB DMA per og), accumulate
    psum[128, n_shard] over the 32 Cin tiles (2 matmuls of N=512 per tile),
    epilogue = single DVE op (+bias into SBUF f32), DMA out.
  * DMA split: x in 8x1MB chunks on sync (HWDGE ring 1); weights on gpsimd
    (SWDGE, 1MB per og, bufs=3 caps early prefetch so the x stream keeps
    most of the shared SDMA bandwidth); y + bias on scalar (HWDGE ring 2,
    keeps the output drain off the x path). og0/og1 are interleaved at
    x-chunk granularity so the PE has ~27.7us of work available while the
    x load completes; the last og runs as two half-width passes so its
    epilogue+store overlaps its own matmuls.
  * Measured: 473us on 8 trn2 cores (vs 1037us staged baseline, 2.19x);
    tensor engine ~93% busy at the bf16 roofline (2048 matmuls x 216ns),
    remaining ~30us = fixed NEFF bootstrap + first-data latency + drain.
  * Tensor work: 2048 matmuls @ N=512 bf16 ~= 443us/core; everything else
    overlaps behind it.
"""

import os
import sys

sys.path.insert(0, "/opt/trn_rl_repo")

from contextlib import ExitStack

import numpy as np
import ml_dtypes

import concourse.bass as bass
import concourse.mybir as mybir
from concourse import bacc
import concourse.tile as tile

F32 = mybir.dt.float32
BF16 = mybir.dt.bfloat16

N_FULL, CIN, COUT = 8192, 4096, 4096
NCORES = 8


def build_nc(n_shard=N_FULL // NCORES, cin=CIN, cout=COUT):
    n_ct = cin // 128       # Cin tiles (contraction)
    n_og = cout // 128      # Cout groups
    HB = min(512, n_shard)  # matmul moving-operand width
    NH = n_shard // HB
    XC = min(4, n_ct)       # cts per 1MB x-load chunk
    NXC = n_ct // XC

    nc = bacc.Bacc()
    # x shard in DRAM already in the SBUF-resident layout [128, n_ct, n_shard]:
    # element [p, ct, k] = xT[ct*128 + p, k]; chunk DMAs are then
    # stride-identical on both sides.
    xtb = nc.declare_dram_parameter(
        "xtb", [128, n_ct, n_shard], BF16, isOutput=False
    )
    wh = nc.declare_dram_parameter("wh", [n_og, 128, n_ct * 128], BF16, isOutput=False)
    bp = nc.declare_dram_parameter("bp", [128, n_og], F32, isOutput=False)
    yt = nc.declare_dram_parameter("yt", [cout, n_shard], F32, isOutput=True)

    with tile.TileContext(nc) as tc, ExitStack() as ctx:
        p_xall = ctx.enter_context(tc.tile_pool(name="xall", bufs=1))
        p_w = ctx.enter_context(tc.tile_pool(name="wts", bufs=3))
        p_out = ctx.enter_context(tc.tile_pool(name="outs", bufs=4))
        p_s = ctx.enter_context(tc.tile_pool(name="svec", bufs=1))

        ball = p_s.tile([128, n_og], F32)
        nc.scalar.dma_start(ball, bp[:])

        # x resident in SBUF as bf16, contraction dim on partitions;
        # 1MB chunks on the sync HWDGE ring
        xall = p_xall.tile([128, n_ct, n_shard], BF16)
        for c in range(NXC):
            nc.sync.dma_start(
                xall[:, c * XC:(c + 1) * XC, :],
                xtb[:, c * XC:(c + 1) * XC, :],
            )

        # weights: one 1MB DMA per og on gpsimd SWDGE; bufs=3 caps early
        # prefetch at 3MB so the x stream keeps most of the SDMA bandwidth
        wt_all = []
        for og in range(n_og):
            wt_ = p_w.tile([128, n_ct * 128], BF16, tag="w")
            nc.gpsimd.dma_start(wt_, wh[og, :, :])
            wt_all.append(wt_)

        def mm_og(p_ps, og, cts, ps=None, nbs=range(NH)):
            if ps is None:
                ps = p_ps.tile([128, n_shard], F32, tag="ps")
            for ct in cts:
                lhs = wt_all[og][:, ct * 128:(ct + 1) * 128]
                first, last = ct == 0, ct == n_ct - 1
                for nb in nbs:
                    ns = slice(nb * HB, (nb + 1) * HB)
                    nc.tensor.matmul(
                        ps[:, ns], lhs, xall[:, ct, ns],
                        start=first, stop=last,
                    )
            return ps

        def epilogue(og, ps, nb=None):
            if nb is None:
                t1 = p_out.tile([128, n_shard], F32, tag="t1")
                nc.vector.tensor_scalar_add(t1, ps, ball[:, og:og + 1])
                nc.scalar.dma_start(yt[og * 128:(og + 1) * 128, :], t1)
            else:
                ns = slice(nb * HB, (nb + 1) * HB)
                t1 = p_out.tile([128, HB], F32, tag="t1h")
                nc.vector.tensor_scalar_add(t1, ps[:, ns], ball[:, og:og + 1])
                nc.scalar.dma_start(yt[og * 128:(og + 1) * 128, ns], t1)

        with tc.tile_pool(name="ps", bufs=4, space="PSUM") as p_ps:
            # first pass: og0+og1 interleaved at x-chunk granularity, so the
            # PE has 2 ogs of work to chew on while x streams in
            ps01 = [None, None]
            for c in range(NXC):
                cts = range(c * XC, (c + 1) * XC)
                for og in (0, 1):
                    ps01[og] = mm_og(p_ps, og, cts, ps01[og])
            for og in (0, 1):
                epilogue(og, ps01[og])

            for og in range(2, n_og - 1):
                ps = mm_og(p_ps, og, range(n_ct))
                epilogue(og, ps)

            # last og: two half-width passes so epilogue+store of the first
            # half overlaps the second half's matmuls (shorter kernel tail)
            og = n_og - 1
            ps = p_ps.tile([128, n_shard], F32, tag="ps")
            for nb in range(NH):
                mm_og(p_ps, og, range(n_ct), ps, nbs=(nb,))
                epilogue(og, ps, nb=nb)

    nc.finalize()
    return nc


def _prep_weights(w, n_og, n_ct):
    # [cout, cin] -> [n_og, 128(p=cin sub), n_ct*128(o)] where
    # out[og, p, ct*128+o] = w[og*128+o, ct*128+p]
    cout, cin = w.shape
    wr = w.reshape(n_og, 128, n_ct, 128)        # [og, o, ct, p]
    wr = wr.transpose(0, 3, 2, 1)               # [og, p, ct, o]
    return np.ascontiguousarray(wr.reshape(n_og, 128, n_ct * 128)).astype(
        ml_dtypes.bfloat16
    )


def _prep_svec(v, n_og):
    # [cout] -> [128, n_og] with v[og*128 + p] at [p, og]
    return np.ascontiguousarray(
        np.asarray(v, np.float32).reshape(n_og, 128).T
    )


def kernel(x, w0, w1, s0, s1, bias):
    from concourse.bass_utils import run_bass_kernel_spmd

    N, Cin = x.shape[0], x.shape[1]
    Cout = w0.shape[0]
    n_shard = N // NCORES
    n_ct = Cin // 128
    n_og = Cout // 128

    x2d = np.asarray(x, dtype=np.float32).reshape(N, Cin)
    Wd = (
        np.asarray(s0, np.float32)[:, None] * np.asarray(w0, np.float32)
        + np.asarray(s1, np.float32)[:, None] * np.asarray(w1, np.float32)
    )

    whp = _prep_weights(Wd, n_og, n_ct)
    bpp = _prep_svec(bias, n_og)

    nc = build_nc(n_shard=n_shard, cin=Cin, cout=Cout)

    in_maps = []
    for k in range(NCORES):
        xtk = x2d[k * n_shard:(k + 1) * n_shard].T  # [Cin, n_shard]
        xtk = xtk.reshape(n_ct, 128, n_shard).transpose(1, 0, 2)
        xtk = np.ascontiguousarray(xtk).astype(ml_dtypes.bfloat16)
        in_maps.append({"xtb": xtk, "wh": whp, "bp": bpp})

    res = run_bass_kernel_spmd(
        nc,
        in_maps,
        core_ids=list(range(NCORES)),
        trace=bool(int(os.environ.get("KERNEL_TRACE", "0"))),
    )

    y = np.empty((N, Cout), dtype=np.float32)
    for k in range(NCORES):
        y[k * n_shard:(k + 1) * n_shard] = res.results[k]["yt"].T
    out = y.reshape(N, Cout, 1, 1)
    kernel.last_results = res
    return out


# revision 21
# speedup vs baseline: 1.0035x; 1.0030x over previous
"""Trainium2 Bass kernel for nn_DualInt8LinearConv.

Reference computation (N=8192, Cin=4096, Cout=4096):
    x2d      = x.reshape(N, Cin)
    amax     = max(|x2d|, axis=1)                      # [N]
    scale_x  = softplus(amax) / 32767                  # [N]
    xq       = round(x2d / scale_x)                    # [N, Cin]  (never clipped:
                                                       #  softplus(a) > a => |x|/scale < 32767)
    y        = (xq@w0.T * s0 + xq@w1.T * s1) * scale_x + bias

Algebraic collapse: with Wd = s0[:,None]*w0 + s1[:,None]*w1,
    y = scale_x * (xq @ Wd.T) + bias
      = (x + scale_x*eps) @ Wd.T + bias          (eps = rounding, |eps| <= 0.5)
      ~ x @ Wd.T + bias
The xq rounding term contributes ~4e-5 absolute (vs |y|max ~7.3), i.e. the whole
fake-quant pipeline is a single GEMM to well below the 2e-2 gate. Casting x and
Wd to bf16 gives measured scale-relative error 2.4e-3.

Strategy:
  * Row-shard N across 8 cores (1024 rows each); Wd replicated.
  * Host: Wd in fp32, pre-tiled to [n_og, 128(cin sub), n_ct*128(cout)] bf16;
    x shard pre-transposed to [Cin, n_shard] and cast bf16 (halves DMA).
  * Device: x resident in SBUF [128, n_ct, n_shard] bf16 (64KB/partition).
    Per 128-Cout group og: stream Wd (1MB DMA per og), accumulate
    psum[128, n_shard] over the 32 Cin tiles (2 matmuls of N=512 per tile),
    epilogue = single DVE op (+bias into SBUF f32), DMA out.
  * DMA split: x in 8x1MB chunks on sync (HWDGE ring 1); weights on gpsimd
    (SWDGE, 1MB per og, bufs=3 caps early prefetch so the x stream keeps
    most of the shared SDMA bandwidth); y + bias on scalar (HWDGE ring 2,
    keeps the output drain off the x path). og0/og1 are interleaved at
    x-chunk granularity so the PE has ~27.7us of work available while the
    x load completes; the last og runs as two half-width passes so its
    epilogue+store overlaps its own matmuls.
  * Measured: 473us on 8 trn2 cores (vs 1037us staged baseline, 2.19x);
    tensor engine ~93% busy at the bf16 roofline (2048 matmuls x 216ns),
    remaining ~30us = fixed NEFF bootstrap + first-data latency + drain.
  * Tensor work: 2048 matmuls @ N=512 bf16 ~= 443us/core; everything else
    overlaps behind it.
"""

import os
import sys

sys.path.insert(0, "/opt/trn_rl_repo")

from contextlib import ExitStack

import numpy as np
import ml_dtypes

import concourse.bass as bass
import concourse.mybir as mybir
from concourse import bacc
import concourse.tile as tile

F32 = mybir.dt.float32
BF16 = mybir.dt.bfloat16

N_FULL, CIN, COUT = 8192, 4096, 4096
NCORES = 8


def build_nc(n_shard=N_FULL // NCORES, cin=CIN, cout=COUT):
    n_ct = cin // 128       # Cin tiles (contraction)
    n_og = cout // 128      # Cout groups
    HB = min(512, n_shard)  # matmul moving-operand width
    NH = n_shard // HB
    XC = min(4, n_ct)       # cts per 1MB x-load chunk
    NXC = n_ct // XC

    nc = bacc.Bacc()
    # x shard in DRAM already in the SBUF-resident layout [128, n_ct, n_shard]:
    # element [p, ct, k] = xT[ct*128 + p, k]; chunk DMAs are then
    # stride-identical on both sides.
    xtb = nc.declare_dram_parameter(
        "xtb", [128, n_ct, n_shard], BF16, isOutput=False
    )
    wh = nc.declare_dram_parameter("wh", [n_og, 128, n_ct * 128], BF16, isOutput=False)
    bp = nc.declare_dram_parameter("bp", [128, n_og], F32, isOutput=False)
    yt = nc.declare_dram_parameter("yt", [cout, n_shard], F32, isOutput=True)

    with tile.TileContext(nc) as tc, ExitStack() as ctx:
        p_xall = ctx.enter_context(tc.tile_pool(name="xall", bufs=1))
        p_w = ctx.enter_context(tc.tile_pool(name="wts", bufs=3))
        p_out = ctx.enter_context(tc.tile_pool(name="outs", bufs=4))
        p_s = ctx.enter_context(tc.tile_pool(name="svec", bufs=1))

        ball = p_s.tile([128, n_og], F32)
        nc.scalar.dma_start(ball, bp[:])

        # x resident in SBUF as bf16, contraction dim on partitions, on the
        # sync HWDGE ring. The first chunk is split per-ct (256KB) so the
        # very first matmul only gates on a small transfer — everything
        # before it is fixed preamble — and the rest ride efficient 1MB
        # chunks.
        xall = p_xall.tile([128, n_ct, n_shard], BF16)
        for ct in range(XC):
            nc.sync.dma_start(xall[:, ct, :], xtb[:, ct, :])
        for c in range(1, NXC):
            nc.sync.dma_start(
                xall[:, c * XC:(c + 1) * XC, :],
                xtb[:, c * XC:(c + 1) * XC, :],
            )

        # weights on gpsimd SWDGE; bufs=3 caps early prefetch at 3MB so the
        # x stream keeps most of the SDMA bandwidth. og0/og1 load as 256KB
        # quarters (first-MM latency), the rest as single 1MB DMAs.
        wt_all = []
        for og in range(n_og):
            wt_ = p_w.tile([128, n_ct * 128], BF16, tag="w")
            if og < 2:
                q = n_ct * 128 // 4
                for j in range(4):
                    nc.gpsimd.dma_start(
                        wt_[:, j * q:(j + 1) * q], wh[og, :, j * q:(j + 1) * q]
                    )
            else:
                nc.gpsimd.dma_start(wt_, wh[og, :, :])
            wt_all.append(wt_)

        def mm_og(p_ps, og, cts, ps=None, nbs=range(NH)):
            if ps is None:
                ps = p_ps.tile([128, n_shard], F32, tag="ps")
            for ct in cts:
                lhs = wt_all[og][:, ct * 128:(ct + 1) * 128]
                first, last = ct == 0, ct == n_ct - 1
                for nb in nbs:
                    ns = slice(nb * HB, (nb + 1) * HB)
                    nc.tensor.matmul(
                        ps[:, ns], lhs, xall[:, ct, ns],
                        start=first, stop=last,
                    )
            return ps

        def epilogue(og, ps, nb=None):
            if nb is None:
                t1 = p_out.tile([128, n_shard], F32, tag="t1")
                nc.vector.tensor_scalar_add(t1, ps, ball[:, og:og + 1])
                nc.scalar.dma_start(yt[og * 128:(og + 1) * 128, :], t1)
            else:
                ns = slice(nb * HB, (nb + 1) * HB)
                t1 = p_out.tile([128, HB], F32, tag="t1h")
                nc.vector.tensor_scalar_add(t1, ps[:, ns], ball[:, og:og + 1])
                nc.scalar.dma_start(yt[og * 128:(og + 1) * 128, ns], t1)

        with tc.tile_pool(name="ps", bufs=4, space="PSUM") as p_ps:
            # first pass: og0+og1 interleaved at x-chunk granularity, so the
            # PE has 2 ogs of work to chew on while x streams in
            ps01 = [None, None]
            for c in range(NXC):
                cts = range(c * XC, (c + 1) * XC)
                for og in (0, 1):
                    ps01[og] = mm_og(p_ps, og, cts, ps01[og])
            for og in (0, 1):
                epilogue(og, ps01[og])

            for og in range(2, n_og - 1):
                ps = mm_og(p_ps, og, range(n_ct))
                epilogue(og, ps)

            # last og: two half-width passes so epilogue+store of the first
            # half overlaps the second half's matmuls (shorter kernel tail)
            og = n_og - 1
            ps = p_ps.tile([128, n_shard], F32, tag="ps")
            for nb in range(NH):
                mm_og(p_ps, og, range(n_ct), ps, nbs=(nb,))
                epilogue(og, ps, nb=nb)

    nc.finalize()
    return nc


def _prep_weights(w, n_og, n_ct):
    # [cout, cin] -> [n_og, 128(p=cin sub), n_ct*128(o)] where
    # out[og, p, ct*128+o] = w[og*128+o, ct*128+p]
    cout, cin = w.shape
    wr = w.reshape(n_og, 128, n_ct, 128)        # [og, o, ct, p]
    wr = wr.transpose(0, 3, 2, 1)               # [og, p, ct, o]
    return np.ascontiguousarray(wr.reshape(n_og, 128, n_ct * 128)).astype(
        ml_dtypes.bfloat16
    )


def _prep_svec(v, n_og):
    # [cout] -> [128, n_og] with v[og*128 + p] at [p, og]
    return np.ascontiguousarray(
        np.asarray(v, np.float32).reshape(n_og, 128).T
    )


def kernel(x, w0, w1, s0, s1, bias):
    from concourse.bass_utils import run_bass_kernel_spmd

    N, Cin = x.shape[0], x.shape[1]
    Cout = w0.shape[0]
    n_shard = N // NCORES
    n_ct = Cin // 128
    n_og = Cout // 128

    x2d = np.asarray(x, dtype=np.float32).reshape(N, Cin)
    Wd = (
        np.asarray(s0, np.float32)[:, None] * np.asarray(w0, np.float32)
        + np.asarray(s1, np.float32)[:, None] * np.asarray(w1, np.float32)
    )

    whp = _prep_weights(Wd, n_og, n_ct)
    bpp = _prep_svec(bias, n_og)

    nc = build_nc(n_shard=n_shard, cin=Cin, cout=Cout)

    in_maps = []
    for k in range(NCORES):
        xtk = x2d[k * n_shard:(k + 1) * n_shard].T  # [Cin, n_shard]
        xtk = xtk.reshape(n_ct, 128, n_shard).transpose(1, 0, 2)
        xtk = np.ascontiguousarray(xtk).astype(ml_dtypes.bfloat16)
        in_maps.append({"xtb": xtk, "wh": whp, "bp": bpp})

    res = run_bass_kernel_spmd(
        nc,
        in_maps,
        core_ids=list(range(NCORES)),
        trace=bool(int(os.environ.get("KERNEL_TRACE", "0"))),
    )

    y = np.empty((N, Cout), dtype=np.float32)
    for k in range(NCORES):
        y[k * n_shard:(k + 1) * n_shard] = res.results[k]["yt"].T
    out = y.reshape(N, Cout, 1, 1)
    kernel.last_results = res
    return out
